# revision 19
# baseline (speedup 1.0000x reference)
"""GRU decoder kernel for 8 trn2 NeuronCores.

Strategy:
  - GRU scan (T=64, B=64, H=1024) replicated on all 8 cores (batch-64 PE
    matmuls are far more efficient than batch-8 data-parallel ones, and
    replication removes all cross-core communication).
  - Output projection (4096 tokens x 32000 vocab) sharded by vocab: each
    core computes its 4000-column slice against its W_out shard, in
    [vocab, token] layout; host transposes/concats at the end.
  - Precision split: the n-gate (tanh) path and the recurrence run on
    f32r matmuls (~1e-4 matmul rel err at full PE rate); the saturating
    r,z gates and the output projection run bf16 (halves SBUF for W_hh
    and W_out and lets everything stay resident/streamed cheaply).
  - The projection is interleaved into the scan (one 512-token block per
    8 steps) to fill PE gaps left by the per-step gate/elementwise tail,
    and phase 1 (embedding gather + xW precompute) overlaps the early
    scan steps via per-chunk DRAM spill tiles.

Per-step scan math (identical to the reference GRU):
  PSUM[:, 0:2048]  = xw_rz + b_ih_rz + b_hh_rz + h @ W_hh_rz.T   (PE)
  PSUM[:, 2048:]   = b_hh_n + h @ W_hh_n.T                       (PE)
  r, z = sigmoid(PSUM rz)                (ACT)
  v = 1 - z; u = z*h                     (ACT, DVE)
  n = tanh(xw_n + b_ih_n + r * PSUM_n)   (DVE, ACT)
  h' = u + v*n                           (DVE)
  hT' = transpose(h')                    (PE + ACT evict, f32r)
  hs_ring[t%8] <- hT' (bf16)             (DVE; doubles as the r,z lhsT)
"""

import sys

for p in ("/opt/trn_rl_repo", "/root/.axon_site/_ro/trn_rl_repo"):
    if p not in sys.path:
        sys.path.insert(0, p)

import numpy as np
import ml_dtypes

import concourse.bass as bass
import concourse.mybir as mybir
from concourse.tile import TileContext
from concourse import bass_utils

F32 = mybir.dt.float32
F32R = mybir.dt.float32r
BF16 = mybir.dt.bfloat16
I32 = mybir.dt.int32

V, E, H, B, T_FULL = 32000, 512, 1024, 64, 64
G = 3 * H  # 3072
H2 = 2 * H  # 2048
N_CORES = 8
VS = V // N_CORES  # 4000
VP = 4096  # padded vocab shard
SOS_IDX = 1
AF = mybir.ActivationFunctionType


def split_multi_waits(nc):
    """Walrus in this container supports one sync wait per instruction.
    Split multi-wait instructions into single-wait NoOp chains."""
    fn = nc.m.functions[0]
    n_split = 0
    for bb in fn.blocks:
        new = []
        for inst in bb.instructions:
            si = inst.sync_info
            if si is not None and si.on_wait is not None and len(si.on_wait) > 1:
                waits = list(si.on_wait)
                for k, w in enumerate(waits[:-1]):
                    nop = mybir.InstNoOp(name=f"{inst.name}-ws{k}", ins=[], outs=[])
                    nop.engine = inst.engine
                    nop.sync_info = mybir.SyncInfo(on_wait=[w], on_update=[])
                    new.append(nop)
                    n_split += 1
                si.on_wait = [waits[-1]]
                inst.sync_info = si
            new.append(inst)
        bb.instructions = new
    return n_split


def build_nc(T=T_FULL):
    assert T % 8 == 0
    TB = T * B  # tokens
    KC = TB // 128  # phase-1 token chunks
    NPROJ = TB // 512  # projection token blocks

    nc = bass.Bass(trn_type="TRN2")

    idx_d = nc.dram_tensor("idx", [KC, 128, 1], I32, kind="ExternalInput")
    emb_d = nc.dram_tensor("emb", [V, E], F32, kind="ExternalInput")
    wih_d = nc.dram_tensor("wihT", [128, 4, G], F32R, kind="ExternalInput")
    whhrz_d = nc.dram_tensor("whhrzT", [128, 8, H2], BF16, kind="ExternalInput")
    whhn_d = nc.dram_tensor("whhnT", [128, 8, H], F32R, kind="ExternalInput")
    xwbrz_d = nc.dram_tensor("xwbias_rz", [1, H2], BF16, kind="ExternalInput")
    xwbn_d = nc.dram_tensor("xwbias_n", [1, H], F32R, kind="ExternalInput")
    bhn_d = nc.dram_tensor("bhhn", [1, H], F32R, kind="ExternalInput")
    h0t_d = nc.dram_tensor("h0T", [128, 512], F32R, kind="ExternalInput")
    h0tbf_d = nc.dram_tensor("h0Tbf", [128, 8, 64], BF16, kind="ExternalInput")
    h0b_d = nc.dram_tensor("h0bh", [B, H], F32, kind="ExternalInput")
    wout_d = nc.dram_tensor("woutb", [32, 128, 8, 128], BF16, kind="ExternalInput")
    bout_d = nc.dram_tensor("bout", [128, 32], F32, kind="ExternalInput")
    id128_d = nc.dram_tensor("ident128", [128, 128], F32, kind="ExternalInput")
    id64bf_d = nc.dram_tensor("ident64bf", [64, 64], BF16, kind="ExternalInput")
    onesbf_d = nc.dram_tensor("onesbf", [1, 128], BF16, kind="ExternalInput")
    onesr_d = nc.dram_tensor("onesr", [1, 128], F32R, kind="ExternalInput")

    logT_d = nc.dram_tensor("logitsT", [VP, TB], F32, kind="ExternalOutput")
    hout_d = nc.dram_tensor("h_out", [B, H], F32, kind="ExternalOutput")

    with TileContext(nc) as tc:
        with (
            tc.tile_pool(name="const", bufs=1) as constp,
            tc.tile_pool(name="dram", bufs=1, space="DRAM") as dramp,
            tc.tile_pool(name="sxwrz", bufs=2) as sxwrz,
            tc.tile_pool(name="sxwn", bufs=1) as sxwn,
            tc.tile_pool(name="sh", bufs=2) as shp,
            tc.tile_pool(name="shT", bufs=2) as shTp,
            tc.tile_pool(name="sga", bufs=1) as sga,
            tc.tile_pool(name="hsr", bufs=2) as hsrp,
            tc.tile_pool(name="wblk", bufs=2) as wblkp,
            tc.tile_pool(name="osb", bufs=1) as osbp,
            tc.tile_pool(name="spg", bufs=1, space="PSUM") as spg,
            tc.tile_pool(name="pstc", bufs=2, space="PSUM") as pstc,
        ):
            id128 = constp.tile([128, 128], F32)
            nc.sync.dma_start(out=id128[:], in_=id128_d[:])
            id64bf = constp.tile([64, 64], BF16)
            nc.sync.dma_start(out=id64bf[:], in_=id64bf_d[:])
            onesbf = constp.tile([1, 128], BF16)
            nc.sync.dma_start(out=onesbf[:], in_=onesbf_d[:])
            onesr = constp.tile([1, 128], F32R)
            nc.sync.dma_start(out=onesr[:], in_=onesr_d[:])
            bhhn = constp.tile([1, H], F32R)
            nc.sync.dma_start(out=bhhn[:], in_=bhn_d[:])
            bout = constp.tile([128, 32], F32)
            nc.sync.dma_start(out=bout[:], in_=bout_d[:])
            whhrz = constp.tile([128, 8, H2], BF16)
            nc.sync.dma_start(out=whhrz[:], in_=whhrz_d[:])
            whhn = constp.tile([128, 8, H], F32R)
            nc.sync.dma_start(out=whhn[:], in_=whhn_d[:])
            h0tbf = constp.tile([128, 8, 64], BF16)
            nc.sync.dma_start(out=h0tbf[:], in_=h0tbf_d[:])

            xwrz_tiles = [
                dramp.tile([128, H2], BF16, name=f"xwrzd{k}", tag=f"xwrzd{k}")
                for k in range(KC)
            ]
            xwn_tiles = [
                dramp.tile([128, H], F32R, name=f"xwnd{k}", tag=f"xwnd{k}")
                for k in range(KC)
            ]

            # ---------------- Phase 1: gather + transpose + xW ----------------
            with (
                tc.tile_pool(name="wih", bufs=1) as wihp,
                tc.tile_pool(name="p1sb", bufs=2) as p1sb,
                tc.tile_pool(name="p1xt", bufs=2) as p1xt,
                tc.tile_pool(name="p1ev", bufs=2) as p1ev,
            ):
                wih = wihp.tile([128, 4, G], F32R)
                nc.sync.dma_start(out=wih[:], in_=wih_d[:])
                xwbrz = wihp.tile([1, H2], BF16)
                nc.sync.dma_start(out=xwbrz[:], in_=xwbrz_d[:])
                xwbn = wihp.tile([1, H], F32R)
                nc.sync.dma_start(out=xwbn[:], in_=xwbn_d[:])

                for k in range(KC):
                    idx_t = p1sb.tile([128, 1], I32, tag="idx")
                    nc.sync.dma_start(out=idx_t[:], in_=idx_d[k])
                    g_t = p1sb.tile([128, E], F32, tag="gather")
                    nc.gpsimd.indirect_dma_start(
                        out=g_t[:],
                        out_offset=None,
                        in_=emb_d[:],
                        in_offset=bass.IndirectOffsetOnAxis(ap=idx_t[:, :1], axis=0),
                    )
                    px = pstc.tile([128, 512], F32, tag="ps512")
                    for j in range(4):
                        nc.tensor.transpose(
                            out=px[:, j * 128 : (j + 1) * 128],
                            in_=g_t[:, j * 128 : (j + 1) * 128],
                            identity=id128[:],
                        )
                    xT = p1xt.tile([128, 4, 128], F32R)
                    nc.scalar.copy(xT[:], px[:].rearrange("p (j t) -> p j t", j=4))
                    for nch in range(6):
                        pw = pstc.tile([128, 512], F32, tag="ps512")
                        if nch < 4:
                            nc.tensor.matmul(
                                out=pw[:],
                                lhsT=onesbf[:1, :128],
                                rhs=xwbrz[:1, nch * 512 : (nch + 1) * 512],
                                start=True,
                                stop=False,
                            )
                        else:
                            nc.tensor.matmul(
                                out=pw[:],
                                lhsT=onesr[:1, :128],
                                rhs=xwbn[:1, (nch - 4) * 512 : (nch - 3) * 512],
                                start=True,
                                stop=False,
                            )
                        for j in range(4):
                            nc.tensor.matmul(
                                out=pw[:],
                                lhsT=xT[:, j, :],
                                rhs=wih[:, j, nch * 512 : (nch + 1) * 512],
                                start=False,
                                stop=(j == 3),
                            )
                        if nch < 4:
                            ev = p1ev.tile([128, 512], BF16, tag="evrz")
                            nc.scalar.copy(ev[:], pw[:])
                            nc.sync.dma_start(
                                out=xwrz_tiles[k][:, nch * 512 : (nch + 1) * 512],
                                in_=ev[:],
                            )
                        else:
                            ev = p1ev.tile([128, 512], F32R, tag="evn")
                            nc.scalar.copy(ev[:], pw[:])
                            nc.sync.dma_start(
                                out=xwn_tiles[k][:, (nch - 4) * 512 : (nch - 3) * 512],
                                in_=ev[:],
                            )

            # ---------------- Phase 2: scan with interleaved projection -------
            hT_cur = shTp.tile([128, 512], F32R, tag="hT")
            nc.sync.dma_start(out=hT_cur[:], in_=h0t_d[:])
            h_cur = shp.tile([B, H], F32, tag="h")
            nc.sync.dma_start(out=h_cur[:], in_=h0b_d[:])

            hsr = None
            hsr_prev = None
            for t in range(T):
                if t % 8 == 0:
                    hsr_prev = hsr
                    hsr = hsrp.tile([128, 8, 8, 64], BF16, tag="hsr")

                xwrz_t = sxwrz.tile([B, H2], BF16, tag="xwrz")
                nc.sync.dma_start(
                    out=xwrz_t[:],
                    in_=xwrz_tiles[t // 2][(t % 2) * B : (t % 2 + 1) * B, :],
                )
                xwn_t = sxwn.tile([B, H], F32R, tag="xwn")
                nc.sync.dma_start(
                    out=xwn_t[:],
                    in_=xwn_tiles[t // 2][(t % 2) * B : (t % 2 + 1) * B, :],
                )

                # bf16 stationary for the r,z matmuls: previous step's hs ring
                # slot (or h0 at t=0)
                if t == 0:
                    hbf = h0tbf
                elif t % 8 == 0:
                    hbf = hsr_prev[:, 7, :, :]
                else:
                    hbf = hsr[:, t % 8 - 1, :, :]

                PG = spg.tile([B, G], F32)
                # r,z gate columns (banks 0..3, bf16): xw inject + 8 h-chunks
                for nch in range(4):
                    sl = slice(nch * 512, (nch + 1) * 512)
                    nc.tensor.matmul(
                        out=PG[:, sl], lhsT=id64bf[:], rhs=xwrz_t[:, sl],
                        start=True, stop=False,
                    )
                    for c in range(8):
                        nc.tensor.matmul(
                            out=PG[:, sl],
                            lhsT=hbf[:, c, :],
                            rhs=whhrz[:, c, sl],
                            start=False, stop=(c == 7),
                        )
                r_t = sga.tile([B, H], F32, tag="r")
                nc.scalar.activation(r_t[:], PG[:, 0:H], AF.Sigmoid)
                z_t = sga.tile([B, H], F32, tag="z")
                nc.scalar.activation(z_t[:], PG[:, H:H2], AF.Sigmoid)
                v_t = sga.tile([B, H], F32, tag="v")
                nc.scalar.activation(v_t[:], z_t[:], AF.Copy, bias=1.0, scale=-1.0)
                u_t = sga.tile([B, H], F32, tag="u")
                nc.vector.tensor_mul(u_t[:], z_t[:], h_cur[:])

                # n gate columns (banks 4,5, f32r): b_hh inject + 8 h-chunks
                for nch in range(2):
                    sl = slice(H2 + nch * 512, H2 + (nch + 1) * 512)
                    bsl = slice(nch * 512, (nch + 1) * 512)
                    nc.tensor.matmul(
                        out=PG[:, sl], lhsT=onesr[:1, :64], rhs=bhhn[:1, bsl],
                        start=True, stop=False,
                    )
                    for c in range(8):
                        nc.tensor.matmul(
                            out=PG[:, sl],
                            lhsT=hT_cur[:, c * 64 : (c + 1) * 64],
                            rhs=whhn[:, c, bsl],
                            start=False, stop=(c == 7),
                        )
                t1 = sga.tile([B, H], F32, tag="t1")
                nc.vector.tensor_mul(t1[:], r_t[:], PG[:, H2:G])
                npre = sga.tile([B, H], F32, tag="r")
                nc.vector.tensor_add(npre[:], t1[:], xwn_t[:].bitcast(F32))
                n_t = sga.tile([B, H], F32, tag="z")
                nc.scalar.activation(n_t[:], npre[:], AF.Tanh)
                w_t = sga.tile([B, H], F32, tag="t1")
                nc.vector.tensor_mul(w_t[:], v_t[:], n_t[:])
                h_new = shp.tile([B, H], F32, tag="h")
                nc.vector.tensor_add(h_new[:], u_t[:], w_t[:])

                tp = pstc.tile([128, 512], F32, tag="ps512")
                for c in range(8):
                    nc.tensor.transpose(
                        out=tp[:, c * 64 : (c + 1) * 64],
                        in_=h_new[:, c * 128 : (c + 1) * 128],
                        identity=id128[:64, :64],
                    )
                hT_new = shTp.tile([128, 512], F32R, tag="hT")
                nc.scalar.copy(hT_new[:], tp[:])
                nc.vector.tensor_copy(
                    hsr[:, t % 8, :, :],
                    tp[:].rearrange("p (c b) -> p c b", c=8),
                )

                h_cur = h_new
                hT_cur = hT_new

                # ---- projection for the completed 512-token block ----
                if t % 8 == 7:
                    nblk = t // 8
                    for m in range(32):
                        wblk = wblkp.tile([128, 8, 128], BF16, tag="wblk")
                        nc.sync.dma_start(out=wblk[:], in_=wout_d[m])
                        pp = pstc.tile([128, 512], F32, tag="ps512")
                        for c in range(8):
                            nc.tensor.matmul(
                                out=pp[:],
                                lhsT=wblk[:, c, :],
                                rhs=hsr[:, :, c, :],
                                start=(c == 0),
                                stop=(c == 7),
                            )
                        osb = osbp.tile([128, 512], F32, tag="osb")
                        nc.scalar.activation(
                            osb[:], pp[:], AF.Identity, bias=bout[:, m : m + 1]
                        )
                        nc.sync.dma_start(
                            out=logT_d[
                                m * 128 : (m + 1) * 128,
                                nblk * 512 : (nblk + 1) * 512,
                            ],
                            in_=osb[:],
                        )

            nc.sync.dma_start(out=hout_d[:], in_=h_cur[:])

    split_multi_waits(nc)
    return nc


_NC_CACHE = {}


def get_nc(T=T_FULL):
    if T not in _NC_CACHE:
        _NC_CACHE[T] = build_nc(T)
    return _NC_CACHE[T]


def make_host_inputs(encode_vec, targets, emb, W_ih, W_hh, b_ih, b_hh, W_out, b_out, T=T_FULL):
    """Build the per-core input maps (host-side sharding / layout prep)."""
    f32 = np.float32
    bf16 = ml_dtypes.bfloat16
    TB = T * B
    KC = TB // 128

    inputs_tf = np.concatenate(
        [np.full((1, B), SOS_IDX, dtype=targets.dtype), np.asarray(targets)[: T - 1]],
        axis=0,
    )  # [T, B]
    idx = np.ascontiguousarray(inputs_tf.reshape(TB).astype(np.int32)).reshape(KC, 128, 1)

    wihT = np.ascontiguousarray(
        np.asarray(W_ih, f32).T.reshape(4, 128, G).transpose(1, 0, 2)
    )  # [128, 4, G]
    whhT = np.ascontiguousarray(
        np.asarray(W_hh, f32).T.reshape(8, 128, G).transpose(1, 0, 2)
    )  # [128, 8, G]
    whhrz = np.ascontiguousarray(whhT[:, :, :H2]).astype(bf16)
    whhn = np.ascontiguousarray(whhT[:, :, H2:])

    xwbias = np.asarray(b_ih, f32).copy()
    xwbias[:H2] += np.asarray(b_hh, f32)[:H2]
    xwb_rz = xwbias[:H2].reshape(1, H2).astype(bf16)
    xwb_n = np.ascontiguousarray(xwbias[H2:].reshape(1, H))
    bhhn = np.ascontiguousarray(np.asarray(b_hh, f32)[H2:].reshape(1, H))

    h0 = np.asarray(encode_vec, f32)[0]  # [B, H]
    h0T = np.ascontiguousarray(
        h0.T.reshape(8, 128, B).transpose(1, 0, 2).reshape(128, 512)
    )
    h0Tbf = h0T.reshape(128, 8, 64).astype(bf16)

    common = dict(
        idx=idx,
        emb=np.asarray(emb, f32),
        wihT=wihT,
        whhrzT=whhrz,
        whhnT=whhn,
        xwbias_rz=xwb_rz,
        xwbias_n=xwb_n,
        bhhn=bhhn,
        h0T=h0T,
        h0Tbf=h0Tbf,
        h0bh=np.ascontiguousarray(h0),
        ident128=np.eye(128, dtype=f32),
        ident64bf=np.eye(64, dtype=f32).astype(bf16),
        onesbf=np.ones((1, 128), dtype=f32).astype(bf16),
        onesr=np.ones((1, 128), dtype=f32),
    )

    W_out = np.asarray(W_out, f32)
    b_out = np.asarray(b_out, f32)
    in_maps = []
    for c in range(N_CORES):
        wo = W_out[c * VS : (c + 1) * VS]  # [4000, 1024]
        wo_pad = np.zeros((VP, H), dtype=f32)
        wo_pad[:VS] = wo
        blocks = np.ascontiguousarray(
            wo_pad.T.reshape(8, 128, 32, 128).transpose(2, 1, 0, 3)
        ).astype(bf16)  # [32, 128, 8, 128]
        bo_pad = np.zeros((VP,), dtype=f32)
        bo_pad[:VS] = b_out[c * VS : (c + 1) * VS]
        bo = np.ascontiguousarray(bo_pad.reshape(32, 128).T)  # [128, 32]
        m = dict(common)
        m["woutb"] = blocks
        m["bout"] = bo
        in_maps.append(m)
    return in_maps


def assemble_output(results, T=T_FULL):
    out = np.empty((T, B, V), dtype=np.float32)
    for c in range(N_CORES):
        lt = results[c]["logitsT"]  # [VP, TB]
        out[:, :, c * VS : (c + 1) * VS] = lt[:VS].T.reshape(T, B, VS)
    return out


def kernel(encode_vec, targets, emb, W_ih, W_hh, b_ih, b_hh, W_out, b_out):
    nc = get_nc(T_FULL)
    in_maps = make_host_inputs(
        encode_vec, targets, emb, W_ih, W_hh, b_ih, b_hh, W_out, b_out, T=T_FULL
    )
    res = bass_utils.run_bass_kernel_spmd(nc, in_maps, core_ids=list(range(N_CORES)))
    return assemble_output(res.results, T=T_FULL)
